# revision 1
# baseline (speedup 1.0000x reference)
"""GAT (3-layer, PyG-style) Trainium2 Bass kernel, 8-core SPMD.

Sharding: nodes partitioned by dst across 8 cores (2500 each); edges sorted by
dst and assigned to the dst's core, processed per 128-dst tile in 128-edge
chunks. Segment softmax / scatter-add are done with one-hot(dst) matmuls
accumulating in PSUM. Layer 1 gathers x columns from an SBUF-resident x^T via
the gpsimd ap_gather custom op and computes h = x_g @ W on the fly (din=128).
Layers 2/3 compute h on local nodes, AllGather the small h to every core, then
row-gather per edge chunk with indirect DMA ([128,1] offsets — one row per
partition, which is what the HW DGE supports). hd(dst) is broadcast to edges
with the transposed one-hot matmul (dst rows are contiguous per tile, no
gather needed). Softmax skips the segment-max shift (scores are O(1); exp
cannot overflow in fp32).
"""

import os
import sys
from contextlib import ExitStack

import numpy as np

for _p in ("/opt/trn_rl_repo", "/root/.axon_site/_ro/trn_rl_repo"):
    if os.path.isdir(_p) and _p not in sys.path:
        sys.path.insert(0, _p)

from concourse import bass, mybir, tile  # noqa: E402
from concourse import bacc  # noqa: E402
from concourse.bass_utils import run_bass_kernel_spmd  # noqa: E402

P = 128
DIN = 128
NCORES = 8
LEAK = 0.2
EPS = 1e-16
BNE = 1e-5
LAYERS = [(128, 128, 8), (1024, 64, 4), (256, 32, 1)]  # (din, C, H)
F32 = mybir.dt.float32
F32R = mybir.dt.float32r
I32 = mybir.dt.int32
I16 = mybir.dt.int16
AF = mybir.ActivationFunctionType
OP = mybir.AluOpType


# ---------------------------------------------------------------- host side

def _preprocess_edges(edge_index, n, ncores):
    """Sort self-loop-augmented edges by dst, partition by dst core, pad each
    128-dst tile's edges to chunks of 128. k per tile is shared across cores
    (max) so the SPMD program is identical on every core."""
    nloc = n // ncores
    ntiles = (nloc + P - 1) // P
    src = np.concatenate([edge_index[0], np.arange(n, dtype=edge_index.dtype)])
    dst = np.concatenate([edge_index[1], np.arange(n, dtype=edge_index.dtype)])
    order = np.argsort(dst, kind="stable")
    src = src[order].astype(np.int32)
    dst = dst[order].astype(np.int32)

    core_of = dst // nloc
    loc = dst - core_of * nloc
    t_of = loc // P

    counts = np.zeros((ncores, ntiles), dtype=np.int64)
    for c in range(ncores):
        tt = t_of[core_of == c]
        for t in range(ntiles):
            counts[c, t] = int(np.sum(tt == t))
    k_list = [max(1, int(np.ceil(counts[:, t].max() / P))) for t in range(ntiles)]
    nch = int(np.sum(k_list))
    offs = np.cumsum([0] + k_list)[:-1]

    SRC = np.zeros((ncores, P, nch), dtype=np.int32)
    IDX16 = np.zeros((ncores, P, nch * 8), dtype=np.int16)
    DSTF = np.full((ncores, P, nch), 200.0, dtype=np.float32)
    for c in range(ncores):
        sel = core_of == c
        s_c, l_c, t_c = src[sel], loc[sel], t_of[sel]
        for t in range(ntiles):
            m = t_c == t
            s_t, l_t = s_c[m], l_c[m]
            ne = len(s_t)
            k = k_list[t]
            pad = k * P - ne
            s_p = np.concatenate([s_t, np.zeros(pad, dtype=np.int32)])
            d_p = np.concatenate(
                [(l_t - t * P).astype(np.float32), np.full(pad, 200.0, np.float32)]
            )
            # chunk j, partition p  <- edge j*P + p ; table col = offs[t]+j
            SRC[c, :, offs[t]:offs[t] + k] = s_p.reshape(k, P).T
            DSTF[c, :, offs[t]:offs[t] + k] = d_p.reshape(k, P).T
            # ap_gather idx layout: position i of chunk j = idxs[i%16, i//16],
            # replicated across the 8 16-partition groups
            sj = s_p.reshape(k, P)  # [k, 128]
            for j in range(k):
                blk = sj[j].reshape(8, 16)  # [s, p]
                col = (offs[t] + j) * 8
                for g in range(8):
                    IDX16[c, 16 * g:16 * (g + 1), col:col + 8] = \
                        blk.T.astype(np.int16)
    return k_list, SRC, IDX16, DSTF


def _fold_weights(inp):
    """Collapse attention vectors into Ws/Wd, bias+BN into scale/shift."""
    out = {}
    for i, (din, c, h) in enumerate(LAYERS, 1):
        W = np.asarray(inp[f"W{i}"], dtype=np.float32)  # [din, h*c]
        a_s = np.asarray(inp[f"as{i}"], dtype=np.float32)  # [h, c]
        a_d = np.asarray(inp[f"ad{i}"], dtype=np.float32)
        Wr = W.reshape(din, h, c)
        Ws = np.einsum("dhc,hc->dh", Wr, a_s)
        Wd = np.einsum("dhc,hc->dh", Wr, a_d)
        out[f"W{i}"] = W
        out[f"WSD{i}"] = np.ascontiguousarray(
            np.concatenate([Ws, Wd], axis=1))  # [din, 2h]
        g = np.asarray(inp[f"g{i}"], np.float32)
        be = np.asarray(inp[f"be{i}"], np.float32)
        m = np.asarray(inp[f"m{i}"], np.float32)
        v = np.asarray(inp[f"v{i}"], np.float32)
        b = np.asarray(inp[f"b{i}"], np.float32)
        sc = g / np.sqrt(v + BNE)
        sh = be + (b - m) * sc
        out[f"SC{i}"] = np.tile(sc[None, :], (P, 1)).astype(np.float32)
        out[f"SH{i}"] = np.tile(sh[None, :], (P, 1)).astype(np.float32)
    out["WC1"] = np.asarray(inp["Wc1"], np.float32)
    out["WC2"] = np.asarray(inp["Wc2"], np.float32)
    out["BC1"] = np.tile(np.asarray(inp["bc1"], np.float32)[None, :], (P, 1))
    out["BC2"] = np.tile(np.asarray(inp["bc2"], np.float32)[None, :], (P, 1))
    return out


# ---------------------------------------------------------------- device side

def build_kernel(n, ncores, k_list, debug_taps=False):
    nloc = n // ncores
    ntiles = len(k_list)
    offs = np.cumsum([0] + list(k_list))[:-1]
    nch = int(np.sum(k_list))
    kmax = max(k_list)

    nc = bacc.Bacc("TRN2", target_bir_lowering=False, debug=False,
                   num_devices=ncores)

    xt_d = nc.dram_tensor("XT", [P, n], F32, kind="ExternalInput")
    xloct_d = nc.dram_tensor("XLOCT", [P, nloc], F32, kind="ExternalInput")
    src_d = nc.dram_tensor("SRC", [P, nch], I32, kind="ExternalInput")
    idx_d = nc.dram_tensor("IDX16", [P, nch * 8], I16, kind="ExternalInput")
    dstf_d = nc.dram_tensor("DSTF", [P, nch], F32, kind="ExternalInput")
    iota_d = nc.dram_tensor("IOTA", [P, P], F32, kind="ExternalInput")
    ident_d = nc.dram_tensor("IDENT", [P, P], F32, kind="ExternalInput")
    w_d = {}
    for i, (din, c, h) in enumerate(LAYERS, 1):
        f = h * c if i < 3 else c
        w_d[f"W{i}"] = nc.dram_tensor(f"W{i}", [din, h * c], F32, kind="ExternalInput")
        w_d[f"WSD{i}"] = nc.dram_tensor(f"WSD{i}", [din, 2 * h], F32,
                                        kind="ExternalInput")
        w_d[f"SC{i}"] = nc.dram_tensor(f"SC{i}", [P, f], F32, kind="ExternalInput")
        w_d[f"SH{i}"] = nc.dram_tensor(f"SH{i}", [P, f], F32, kind="ExternalInput")
    w_d["WC1"] = nc.dram_tensor("WC1", [32, 16], F32, kind="ExternalInput")
    w_d["WC2"] = nc.dram_tensor("WC2", [16, 2], F32, kind="ExternalInput")
    w_d["BC1"] = nc.dram_tensor("BC1", [P, 16], F32, kind="ExternalInput")
    w_d["BC2"] = nc.dram_tensor("BC2", [P, 2], F32, kind="ExternalInput")

    out_d = nc.dram_tensor("OUT", [nloc, 2], F32, kind="ExternalOutput")
    if debug_taps:
        dbg_x2 = nc.dram_tensor("DBG_X2", [nloc, 1024], F32, kind="ExternalOutput")
        dbg_al = nc.dram_tensor("DBG_AL", [P, kmax * 8], F32, kind="ExternalOutput")
        dbg_sc = nc.dram_tensor("DBG_SC", [P, kmax * 8], F32, kind="ExternalOutput")

    # internal DRAM; indirect-gather sources must be whole tensors (offset 0)
    shared = "Shared" if ncores > 1 else "Local"
    hd1_loc = nc.dram_tensor("hd1_loc", [nloc, 8], F32)
    hd2_loc = nc.dram_tensor("hd2_loc", [nloc, 4], F32)
    hd3_loc = nc.dram_tensor("hd3_loc", [nloc, 1], F32)
    ag2_in = nc.dram_tensor("ag2_in", [nloc, 264], F32)
    ag2_out = nc.dram_tensor("ag2_out", [n, 264], F32, addr_space=shared)
    ag3_in = nc.dram_tensor("ag3_in", [nloc, 34], F32)
    ag3_out = nc.dram_tensor("ag3_out", [n, 34], F32, addr_space=shared)

    def rows_of(t):
        return min(P, nloc - t * P)

    with ExitStack() as ctx:
        tc = ctx.enter_context(tile.TileContext(nc))
        cpool = ctx.enter_context(tc.tile_pool(name="const", bufs=1))

        # ---- constants / weights in SBUF
        iota = cpool.tile([P, P], F32)
        nc.sync.dma_start(out=iota[:], in_=iota_d[:, :])
        ident = cpool.tile([P, P], F32)
        nc.sync.dma_start(out=ident[:], in_=ident_d[:, :])
        identr = cpool.tile([P, P], F32R)
        nc.vector.tensor_copy(out=identr[:], in_=ident[:])
        wsb = {}

        def load_f32r(name, dram_ap, shape):
            tmp = cpool.tile(shape, F32, name=f"{name}tmp")
            nc.sync.dma_start(out=tmp[:], in_=dram_ap)
            t = cpool.tile(shape, F32R, name=f"{name}r")
            nc.vector.tensor_copy(out=t[:], in_=tmp[:])
            return t

        for i, (din, c, h) in enumerate(LAYERS, 1):
            f = h * c if i < 3 else c
            kb = din // P
            if i < 3:
                wsb[f"W{i}"] = load_f32r(
                    f"W{i}sb",
                    w_d[f"W{i}"][:, :].rearrange("(kb p) f -> p kb f", p=P),
                    [P, kb, h * c])
                wsb[f"WSD{i}"] = load_f32r(
                    f"WSD{i}sb",
                    w_d[f"WSD{i}"][:, :].rearrange("(kb p) f -> p kb f", p=P),
                    [P, kb, 2 * h])
            else:
                for nm2, w2 in ((f"W{i}", h * c), (f"WSD{i}", 2 * h)):
                    wsb[nm2] = cpool.tile([P, kb, w2], F32, name=f"{nm2}sb")
                    nc.sync.dma_start(
                        out=wsb[nm2][:],
                        in_=w_d[nm2][:, :].rearrange("(kb p) f -> p kb f", p=P))
            for nm in (f"SC{i}", f"SH{i}"):
                wsb[nm] = cpool.tile([P, f], F32, name=f"{nm}sb")
                nc.sync.dma_start(out=wsb[nm][:], in_=w_d[nm][:, :])
        for nm, shp in (("WC1", [32, 16]), ("WC2", [16, 2]),
                        ("BC1", [P, 16]), ("BC2", [P, 2])):
            wsb[nm] = cpool.tile(shp, F32, name=f"{nm}sb")
            nc.sync.dma_start(out=wsb[nm][:], in_=w_d[nm][:, :])

        # =========== phase 0: hs|hd of layer 1 for local nodes -> hd1_loc
        with tc.tile_pool(name="p0", bufs=2) as pool, \
             tc.tile_pool(name="p0p", bufs=2, space="PSUM") as pp0:
            xloct = pool.tile([P, nloc], F32, tag="xloct")
            nc.sync.dma_start(out=xloct[:], in_=xloct_d[:, :])
            xloctr = pool.tile([P, nloc], F32R, tag="xloctr")
            nc.scalar.copy(out=xloctr[:], in_=xloct[:])
            for t in range(ntiles):
                r = rows_of(t)
                hp = pp0.tile([P, 16], F32, tag="hp")
                nc.tensor.matmul(hp[:r], lhsT=xloctr[:, t * P:t * P + r],
                                 rhs=wsb["WSD1"][:, 0, :], start=True, stop=True)
                hsb = pool.tile([P, 16], F32, tag="hsb")
                nc.vector.tensor_copy(out=hsb[:r], in_=hp[:r])
                nc.sync.dma_start(out=hd1_loc[t * P:t * P + r, :],
                                  in_=hsb[:r, 8:16])

        # =========== generic GAT layer =========== #
        def gat_layer(li, C, H, hg_dram, rowlen, hd_dram, epilogue, xtp=None):
            """hg_dram None -> layer 1: ap_gather x columns from resident x^T
            and compute h on the fly; else row-gather [h | hs] (width rowlen)
            from hg_dram per 128-edge chunk."""
            f = H * C
            nhalf = (f + 511) // 512
            MDT = F32R if H > 1 else F32  # fp32r needs even free counts
            identm = identr if H > 1 else ident
            if hg_dram is None:
                xt = xtp.tile([P, n], F32, tag="xt")
                nc.sync.dma_start(out=xt[:], in_=xt_d[:, :])
            with tc.tile_pool(name=f"L{li}", bufs=2) as pool, \
                 tc.tile_pool(name=f"L{li}g", bufs=3) as poolg, \
                 tc.tile_pool(name=f"L{li}ps", bufs=2, space="PSUM") as pp, \
                 tc.tile_pool(name=f"L{li}acc", bufs=1, space="PSUM") as pacc, \
                 tc.tile_pool(name=f"L{li}sc", bufs=2, space="PSUM") as psc:
                for t in range(ntiles):
                    k = k_list[t]
                    off = int(offs[t])
                    r = rows_of(t)
                    kh16 = ((k * H + 15) // 16) * 16
                    zoff = kh16
                    zboff = kh16 + 16
                    hdoff = zboff + ((k * H + 7) // 8) * 8

                    dstf = pool.tile([P, kmax], F32, tag="dstf")
                    nc.sync.dma_start(out=dstf[:, :k], in_=dstf_d[:, off:off + k])
                    hdt = pool.tile([P, H], F32, tag="hdt")
                    if r < P:
                        nc.vector.memset(hdt[:], 0.0)
                    nc.sync.dma_start(out=hdt[:r], in_=hd_dram[t * P:t * P + r, :])
                    hdtm = pool.tile([P, H], MDT, tag="hdtm")
                    nc.vector.tensor_copy(out=hdtm[:], in_=hdt[:])

                    if hg_dram is None:
                        idx = pool.tile([P, kmax * 8], I16, tag="idx")
                        nc.sync.dma_start(out=idx[:, :k * 8],
                                          in_=idx_d[:, off * 8:(off + k) * 8])
                        xgT = poolg.tile([P, kmax * DIN], F32, tag="xgT")
                        xgTr = poolg.tile([P, kmax * DIN], F32R, tag="xgTr")
                    else:
                        osrc = pool.tile([P, kmax], I32, tag="osrc")
                        nc.sync.dma_start(out=osrc[:, :k],
                                          in_=src_d[:, off:off + k])
                        hg = poolg.tile([P, kmax * rowlen], F32, tag="hg")
                        hg3 = hg[:, :k * rowlen].rearrange(
                            "p (kk d) -> p kk d", d=rowlen)

                    onehot = poolg.tile([P, kmax * P], MDT, tag="onehot")
                    onehotT = poolg.tile([P, kmax * P], MDT, tag="onehotT")
                    # one PSUM bank: [scores | Z | Zbcast | hd_e]
                    pscT = psc.tile([P, hdoff + ((k * H + 7) // 8) * 8], F32,
                                    tag="psc")

                    # --- loop A: onehots, hd broadcast, gathers, scores
                    for j in range(k):
                        js = slice(j * P, (j + 1) * P)
                        nc.vector.tensor_scalar(
                            out=onehot[:, js], in0=iota[:],
                            scalar1=dstf[:, j:j + 1], scalar2=None,
                            op0=OP.is_equal)
                        tp = pp.tile([P, P], MDT, tag="tp")
                        nc.tensor.transpose(tp[:], onehot[:, js], identm[:])
                        nc.scalar.copy(out=onehotT[:, js], in_=tp[:])
                        nc.tensor.matmul(
                            pscT[:, hdoff + j * H:hdoff + (j + 1) * H],
                            lhsT=onehotT[:, js], rhs=hdtm[:],
                            start=True, stop=True)
                        if hg_dram is None:
                            nc.gpsimd.ap_gather(
                                out_ap=xgT[:, j * DIN:(j + 1) * DIN],
                                in_ap=xt[:], idxs_ap=idx[:, j * 8:(j + 1) * 8],
                                channels=P, num_elems=n, d=1, num_idxs=P)
                            nc.scalar.copy(out=xgTr[:, j * DIN:(j + 1) * DIN],
                                           in_=xgT[:, j * DIN:(j + 1) * DIN])
                            nc.tensor.matmul(
                                pscT[:, j * H:(j + 1) * H],
                                lhsT=xgTr[:, j * DIN:(j + 1) * DIN],
                                rhs=wsb[f"WSD{li}"][:, 0, 0:H],
                                start=True, stop=True)
                        else:
                            nc.gpsimd.indirect_dma_start(
                                out=hg[:, j * rowlen:(j + 1) * rowlen],
                                out_offset=None, in_=hg_dram[:, :],
                                in_offset=bass.IndirectOffsetOnAxis(
                                    ap=osrc[:, j:j + 1], axis=0))

                    # --- scores -> exp(leaky(hs + hd))
                    sc = pool.tile([P, kmax * H], F32, tag="sc")
                    if hg_dram is None:
                        hdsb = pool.tile([P, kmax * H], F32, tag="hdsb")
                        nc.scalar.copy(out=hdsb[:, :k * H],
                                       in_=pscT[:, hdoff:hdoff + k * H])
                        nc.vector.tensor_tensor(
                            out=sc[:, :k * H], in0=pscT[:, 0:k * H],
                            in1=hdsb[:, :k * H], op=OP.add)
                    else:
                        nc.vector.tensor_tensor(
                            out=sc[:, :k * H].rearrange("p (kk h) -> p kk h", h=H),
                            in0=hg3[:, :, f:f + H],
                            in1=pscT[:, hdoff:hdoff + k * H].rearrange(
                                "p (kk h) -> p kk h", h=H),
                            op=OP.add)
                    sc2 = pool.tile([P, kmax * H], F32, tag="sc2")
                    nc.vector.tensor_scalar(out=sc2[:, :k * H], in0=sc[:, :k * H],
                                            scalar1=LEAK, scalar2=None, op0=OP.mult)
                    nc.vector.tensor_tensor(out=sc2[:, :k * H], in0=sc[:, :k * H],
                                            in1=sc2[:, :k * H], op=OP.max)
                    expsc = pool.tile([P, kmax * H], MDT, tag="expsc")
                    nc.scalar.activation(out=expsc[:, :k * H], in_=sc2[:, :k * H],
                                         func=AF.Exp)
                    if debug_taps and li == 1 and t == 0:
                        nc.sync.dma_start(out=dbg_sc[:, :k * H], in_=sc[:, :k * H])

                    # --- Z = segment-sum(exp)
                    for j in range(k):
                        nc.tensor.matmul(
                            pscT[:, zoff:zoff + H],
                            lhsT=onehot[:, j * P:(j + 1) * P],
                            rhs=expsc[:, j * H:(j + 1) * H],
                            start=(j == 0), stop=(j == k - 1))
                    zr = pool.tile([P, H], F32, tag="zr")
                    nc.vector.tensor_scalar(out=zr[:], in0=pscT[:, zoff:zoff + H],
                                            scalar1=float(EPS), scalar2=None,
                                            op0=OP.add)
                    zr2 = pool.tile([P, H], MDT, tag="zr2")
                    with nc.allow_low_precision(reason="1/Z bcast via matmul"):
                        nc.vector.reciprocal(out=zr2[:], in_=zr[:])

                    # --- broadcast 1/Z to edges; alpha = exp * (1/Z)[dst]
                    for j in range(k):
                        nc.tensor.matmul(
                            pscT[:, zboff + j * H:zboff + (j + 1) * H],
                            lhsT=onehotT[:, j * P:(j + 1) * P], rhs=zr2[:],
                            start=True, stop=True)
                    alpha = pool.tile([P, kmax * H], F32, tag="alpha")
                    nc.vector.tensor_tensor(
                        out=alpha[:, :k * H], in0=expsc[:, :k * H],
                        in1=pscT[:, zboff:zboff + k * H], op=OP.mult)
                    if debug_taps and li == 1 and t == 0:
                        nc.sync.dma_start(out=dbg_al[:, :k * H],
                                          in_=alpha[:, :k * H])

                    # --- weighted aggregation into PSUM accumulator
                    acc = pacc.tile([P, f], F32, tag="acc")
                    for j in range(k):
                        for q in range(nhalf):
                            q0 = q * 512
                            qw = min(512, f - q0)
                            hq = qw // C  # heads in this half
                            h0 = q0 // C
                            if hg_dram is None:
                                hj = pp.tile([P, 512], F32, tag="hj")
                                nc.tensor.matmul(
                                    hj[:, :qw],
                                    lhsT=xgTr[:, j * DIN:(j + 1) * DIN],
                                    rhs=wsb[f"W{li}"][:, 0, q0:q0 + qw],
                                    start=True, stop=True)
                                hsrc = hj[:, :qw]
                            else:
                                hsrc = hg3[:, j, q0:q0 + qw]
                            v = pool.tile([P, 512], MDT, tag="v")
                            if H > 1:
                                ab = alpha[:, j * H + h0:j * H + h0 + hq]
                                nc.vector.tensor_tensor(
                                    out=v[:, :qw].rearrange(
                                        "p (h c) -> p h c", c=C),
                                    in0=hsrc.rearrange("p (h c) -> p h c", c=C),
                                    in1=ab[:, :, None].to_broadcast([P, hq, C]),
                                    op=OP.mult)
                            else:
                                nc.vector.tensor_scalar(
                                    out=v[:, :qw], in0=hsrc,
                                    scalar1=alpha[:, j:j + 1], scalar2=None,
                                    op0=OP.mult)
                            nc.tensor.matmul(
                                acc[:, q0:q0 + qw],
                                lhsT=onehot[:, j * P:(j + 1) * P],
                                rhs=v[:, :qw],
                                start=(j == 0), stop=(j == k - 1))

                    epilogue(t, r, acc, pool, pp, psc)

        # ---- epilogue 1: x2 = relu(bn(acc)); h2|hs2 -> ag2_in, hd2 -> hd2_loc
        def epi1(t, r, acc, pool, pp, psc):
            x2 = pool.tile([P, 1024], F32, tag="x2")
            nc.vector.tensor_tensor(out=x2[:], in0=acc[:, 0:1024],
                                    in1=wsb["SC1"][:], op=OP.mult)
            nc.vector.tensor_tensor(out=x2[:], in0=x2[:], in1=wsb["SH1"][:],
                                    op=OP.add)
            nc.scalar.activation(out=x2[:], in_=x2[:], func=AF.Relu)
            if debug_taps:
                nc.sync.dma_start(out=dbg_x2[t * P:t * P + r, :], in_=x2[:r])
            h2p = pp.tile([P, 512], F32, tag="hj")
            hsd2p = psc.tile([P, 16], F32, tag="psc")
            for rr in range(8):
                tp = pp.tile([P, P], F32, tag="tp")
                nc.tensor.transpose(tp[:], x2[:, rr * P:(rr + 1) * P], ident[:])
                x2T = pool.tile([P, P], F32R, tag="x2T")
                nc.scalar.copy(out=x2T[:], in_=tp[:])
                nc.tensor.matmul(h2p[:, 0:256], lhsT=x2T[:],
                                 rhs=wsb["W2"][:, rr, :],
                                 start=(rr == 0), stop=(rr == 7))
                nc.tensor.matmul(hsd2p[:, 0:8], lhsT=x2T[:],
                                 rhs=wsb["WSD2"][:, rr, :],
                                 start=(rr == 0), stop=(rr == 7))
            ag2row = pool.tile([P, 264], F32, tag="ag2row")
            nc.scalar.copy(out=ag2row[:, 0:256], in_=h2p[:, 0:256])
            nc.vector.tensor_copy(out=ag2row[:, 256:264], in_=hsd2p[:, 0:8])
            nc.sync.dma_start(out=ag2_in[t * P:t * P + r, :], in_=ag2row[:r])
            nc.sync.dma_start(out=hd2_loc[t * P:t * P + r, :],
                              in_=ag2row[:r, 260:264])

        with tc.tile_pool(name="xtpool", bufs=1) as xtp:
            gat_layer(1, 128, 8, None, None, hd1_loc, epi1, xtp=xtp)

        if ncores > 1:
            nc.gpsimd.collective_compute(
                "AllGather", OP.bypass,
                replica_groups=[list(range(ncores))],
                ins=[ag2_in[:, :]], outs=[ag2_out[:, :]])
        else:
            nc.sync.dma_start(out=ag2_out[:, :], in_=ag2_in[:, :])

        # ---- epilogue 2: x3 = relu(bn(acc)); h3|hs3 -> ag3_in, hd3 -> hd3_loc
        def epi2(t, r, acc, pool, pp, psc):
            x3 = pool.tile([P, 256], F32, tag="x2")
            nc.vector.tensor_tensor(out=x3[:, 0:256], in0=acc[:, 0:256],
                                    in1=wsb["SC2"][:], op=OP.mult)
            nc.vector.tensor_tensor(out=x3[:, 0:256], in0=x3[:, 0:256],
                                    in1=wsb["SH2"][:], op=OP.add)
            nc.scalar.activation(out=x3[:, 0:256], in_=x3[:, 0:256], func=AF.Relu)
            h3p = pp.tile([P, 512], F32, tag="hj")
            hsd3p = psc.tile([P, 16], F32, tag="psc")
            for rr in range(2):
                tp = pp.tile([P, P], F32, tag="tp")
                nc.tensor.transpose(tp[:], x3[:, rr * P:(rr + 1) * P], ident[:])
                x3T = pool.tile([P, P], F32, tag="x2T")
                nc.scalar.copy(out=x3T[:], in_=tp[:])
                nc.tensor.matmul(h3p[:, 0:32], lhsT=x3T[:],
                                 rhs=wsb["W3"][:, rr, :],
                                 start=(rr == 0), stop=(rr == 1))
                nc.tensor.matmul(hsd3p[:, 0:2], lhsT=x3T[:],
                                 rhs=wsb["WSD3"][:, rr, :],
                                 start=(rr == 0), stop=(rr == 1))
            ag3row = pool.tile([P, 34], F32, tag="ag3row")
            nc.vector.tensor_copy(out=ag3row[:, 0:32], in_=h3p[:, 0:32])
            nc.vector.tensor_copy(out=ag3row[:, 32:34], in_=hsd3p[:, 0:2])
            nc.sync.dma_start(out=ag3_in[t * P:t * P + r, :], in_=ag3row[:r])
            nc.sync.dma_start(out=hd3_loc[t * P:t * P + r, :],
                              in_=ag3row[:r, 33:34])

        gat_layer(2, 64, 4, ag2_out, 264, hd2_loc, epi2)

        if ncores > 1:
            nc.gpsimd.collective_compute(
                "AllGather", OP.bypass,
                replica_groups=[list(range(ncores))],
                ins=[ag3_in[:, :]], outs=[ag3_out[:, :]])
        else:
            nc.sync.dma_start(out=ag3_out[:, :], in_=ag3_in[:, :])

        # ---- epilogue 3: bn+relu, classifier, log_softmax
        def epi3(t, r, acc, pool, pp, psc):
            x4 = pool.tile([P, 32], F32, tag="x4")
            nc.vector.tensor_tensor(out=x4[:], in0=acc[:, 0:32],
                                    in1=wsb["SC3"][:], op=OP.mult)
            nc.vector.tensor_tensor(out=x4[:], in0=x4[:], in1=wsb["SH3"][:],
                                    op=OP.add)
            nc.scalar.activation(out=x4[:], in_=x4[:], func=AF.Relu)
            tp = pp.tile([P, P], F32, tag="tp")
            nc.tensor.transpose(tp[0:32, :], x4[:], ident[:])
            x4T = pool.tile([32, P], F32, tag="x4T")
            nc.scalar.copy(out=x4T[:], in_=tp[0:32, :])
            z1p = psc.tile([P, 16], F32, tag="psc")
            nc.tensor.matmul(z1p[:, 0:16], lhsT=x4T[:], rhs=wsb["WC1"][:],
                             start=True, stop=True)
            z1 = pool.tile([P, 16], F32, tag="z1")
            nc.vector.tensor_tensor(out=z1[:], in0=z1p[:, 0:16], in1=wsb["BC1"][:],
                                    op=OP.add)
            nc.scalar.activation(out=z1[:], in_=z1[:], func=AF.Relu)
            tp2 = pp.tile([P, P], F32, tag="tp")
            nc.tensor.transpose(tp2[0:16, :], z1[:], ident[:])
            z1T = pool.tile([16, P], F32, tag="z1T")
            nc.scalar.copy(out=z1T[:], in_=tp2[0:16, :])
            z2p = psc.tile([P, 16], F32, tag="psc")
            nc.tensor.matmul(z2p[:, 0:2], lhsT=z1T[:], rhs=wsb["WC2"][:],
                             start=True, stop=True)
            z2 = pool.tile([P, 2], F32, tag="z2")
            nc.vector.tensor_tensor(out=z2[:], in0=z2p[:, 0:2], in1=wsb["BC2"][:],
                                    op=OP.add)
            ez = pool.tile([P, 2], F32, tag="ez")
            nc.scalar.activation(out=ez[:], in_=z2[:], func=AF.Exp)
            lse = pool.tile([P, 1], F32, tag="lse")
            nc.vector.reduce_sum(out=lse[:], in_=ez[:], axis=mybir.AxisListType.X)
            nc.scalar.activation(out=lse[:], in_=lse[:], func=AF.Ln)
            res = pool.tile([P, 2], F32, tag="res")
            nc.vector.tensor_scalar(out=res[:], in0=z2[:], scalar1=lse[:, 0:1],
                                    scalar2=None, op0=OP.subtract)
            nc.sync.dma_start(out=out_d[t * P:t * P + r, :], in_=res[:r])

        gat_layer(3, 32, 1, ag3_out, 34, hd3_loc, epi3)

    nc.compile()
    return nc


# ---------------------------------------------------------------- entry point

_CACHE = {}


def make_in_maps(inputs, ncores=NCORES):
    edge_index = np.asarray(inputs["edge_index"])
    x = np.asarray(inputs["x"], dtype=np.float32)
    n = x.shape[0]
    k_list, SRC, IDX16, DSTF = _preprocess_edges(edge_index, n, ncores)
    w = _fold_weights(inputs)

    iota = np.ascontiguousarray(
        np.tile(np.arange(P, dtype=np.float32)[None, :], (P, 1)))
    ident = np.ascontiguousarray(np.eye(P, dtype=np.float32))
    base = dict(XT=np.ascontiguousarray(x.T), IOTA=iota, IDENT=ident)
    for nm in ("W1", "WSD1", "SC1", "SH1", "W2", "WSD2", "SC2", "SH2",
               "W3", "WSD3", "SC3", "SH3", "WC1", "WC2", "BC1", "BC2"):
        base[nm] = np.ascontiguousarray(w[nm])
    nloc = n // ncores
    in_maps = []
    for c in range(ncores):
        m = dict(base)
        m["XLOCT"] = np.ascontiguousarray(x[c * nloc:(c + 1) * nloc].T)
        m["SRC"] = np.ascontiguousarray(SRC[c])
        m["IDX16"] = np.ascontiguousarray(IDX16[c])
        m["DSTF"] = np.ascontiguousarray(DSTF[c])
        in_maps.append(m)
    return n, k_list, in_maps


def kernel(**inputs):
    n, k_list, in_maps = make_in_maps(inputs)
    key = (n, tuple(k_list))
    if key not in _CACHE:
        _CACHE[key] = build_kernel(n, NCORES, k_list)
    nc = _CACHE[key]
    res = run_bass_kernel_spmd(nc, in_maps, core_ids=list(range(NCORES)))
    out = np.concatenate([r["OUT"] for r in res.results], axis=0)
    return out.astype(np.float32)



# revision 23
# speedup vs baseline: 1.2766x; 1.2766x over previous
"""GAT (3-layer, PyG-style) Trainium2 Bass kernel, 8-core SPMD — v2.

Dst-major CSR layout: nodes partitioned by dst across 8 cores (2500 each),
degree-sorted within each core so 128-dst tiles have tight max-degree k_t.
Edge slot (p, j) of a tile = j-th in-edge of the dst at tile row p, so the
chunk-j "one-hot" is the identity: segment softmax is per-partition free-axis
vector work, and scatter-add is matmul(acc, lhsT=identity, rhs=v_j) with PSUM
accumulation. Padded slots point at a sentinel node whose hs = -1e3, so
alpha underflows to exactly 0 (no masks).

Layer 1 ap_gathers x columns from SBUF-resident x^T (f32r end-to-end, no
casts) and computes h|hs per 128-edge chunk. Layers 2/3 gather [h|hs] rows
from the AllGather-ed tables with ONE gpsimd dma_gather per tile (k*128 rows
per instruction), rows padded to 320/64 floats for the 256B-multiple rule.
hd stays in SBUF between layers. Layer-3 aggregation runs entirely on the
vector engine (strided reduce). Output rows are perm-ordered; the host
unpermutes.
"""

import os
import sys
from contextlib import ExitStack

import numpy as np

for _p in ("/opt/trn_rl_repo", "/root/.axon_site/_ro/trn_rl_repo"):
    if os.path.isdir(_p) and _p not in sys.path:
        sys.path.insert(0, _p)

from concourse import bass, mybir, tile  # noqa: E402
from concourse import bacc  # noqa: E402
from concourse.bass_utils import run_bass_kernel_spmd  # noqa: E402

P = 128
DIN = 128
NCORES = 8
LEAK = 0.2
EPS = 1e-16
BNE = 1e-5
F32 = mybir.dt.float32
F32R = mybir.dt.float32r
I16 = mybir.dt.int16
AF = mybir.ActivationFunctionType
OP = mybir.AluOpType
AX = mybir.AxisListType

ROW2 = 320   # layer-2 gather row: [h2(256) | hs2(4) | pad], 1280B (256B mult)
ROW3 = 64    # layer-3 gather row: [h3(32) | hs3(1) | pad], 256B
SENT_HS = -1000.0


# ---------------------------------------------------------------- host side

def _pack_idx16(flat):
    """gather position i <- idxs[i%16, i//16], replicated across the 8
    16-partition groups. flat: [m*128] -> [128, m*8]."""
    cols = len(flat) // 16
    out = np.zeros((P, cols), dtype=np.int16)
    blk = flat.reshape(cols, 16).T  # [16, cols]
    for g in range(8):
        out[16 * g:16 * (g + 1), :] = blk
    return out


def _preprocess(edge_index, n, ncores):
    nloc = n // ncores
    ntiles = (nloc + P - 1) // P
    ei = np.asarray(edge_index)
    src = np.concatenate([ei[0], np.arange(n, dtype=ei.dtype)]).astype(np.int64)
    dst = np.concatenate([ei[1], np.arange(n, dtype=ei.dtype)]).astype(np.int64)
    order = np.argsort(dst, kind="stable")
    src = src[order].astype(np.int32)
    dst = dst[order].astype(np.int32)
    deg = np.bincount(dst, minlength=n).astype(np.int64)
    starts = np.concatenate([[0], np.cumsum(deg)[:-1]])

    # degree-sort each core's local nodes (descending)
    perms, invs = [], []
    for c in range(ncores):
        dc = deg[c * nloc:(c + 1) * nloc]
        p = np.argsort(-dc, kind="stable")
        inv = np.empty(nloc, dtype=np.int64)
        inv[p] = np.arange(nloc)
        perms.append(p)
        invs.append(inv)

    # shared k per tile = max over cores
    k_list = []
    for t in range(ntiles):
        k = 1
        for c in range(ncores):
            rows = perms[c][t * P:(t + 1) * P]
            k = max(k, int(deg[c * nloc + rows].max()))
        k_list.append(k)
    offs = np.cumsum([0] + k_list)[:-1]
    nch = int(np.sum(k_list))

    # device row of original node g (for layer-2/3 gathers); each core's
    # AllGather contribution is nloc+1 rows (last = its sentinel row)
    devrow = np.empty(n, dtype=np.int64)
    outrow = np.empty(n, dtype=np.int64)
    for c in range(ncores):
        devrow[c * nloc:(c + 1) * nloc] = c * (nloc + 1) + invs[c]
        outrow[c * nloc:(c + 1) * nloc] = c * nloc + invs[c]

    # slot tables
    IDX1 = np.zeros((ncores, P, nch * 8), dtype=np.int16)   # ap_gather (x cols)
    IDXG = np.zeros((ncores, P, nch * 8), dtype=np.int16)   # dma_gather rows
    for c in range(ncores):
        for t in range(ntiles):
            k = k_list[t]
            off = int(offs[t])
            rows = perms[c][t * P:(t + 1) * P]  # local orig ids, len <= 128
            g = c * nloc + rows
            srccol = np.full((P, k), n, dtype=np.int64)     # sentinel x col
            srcrow = np.full((P, k), nloc, dtype=np.int64)  # core-0 sentinel
            idx2 = starts[g][:, None] + np.arange(k)[None, :]
            m = np.arange(k)[None, :] < deg[g][:, None]
            vals = src[np.clip(idx2, 0, len(src) - 1)]
            srccol[:len(g)][m] = vals[m]
            srcrow[:len(g)][m] = devrow[vals[m]]
            # dma_gather: position i = j*128+p
            IDXG[c, :, off * 8:(off + k) * 8] = _pack_idx16(
                srcrow.T.ravel().astype(np.int16))
            # ap_gather: per chunk j, position q = column q
            for j in range(k):
                IDX1[c, :, (off + j) * 8:(off + j + 1) * 8] = _pack_idx16(
                    srccol[:, j].astype(np.int16))
    return k_list, IDX1, IDXG, perms, outrow


def _fold_weights(inp):
    out = {}
    layers = [(128, 128, 8), (1024, 64, 4), (256, 32, 1)]
    for i, (din, c, h) in enumerate(layers, 1):
        W = np.asarray(inp[f"W{i}"], dtype=np.float32)
        a_s = np.asarray(inp[f"as{i}"], dtype=np.float32)
        a_d = np.asarray(inp[f"ad{i}"], dtype=np.float32)
        Wr = W.reshape(din, h, c)
        Ws = np.einsum("dhc,hc->dh", Wr, a_s)
        Wd = np.einsum("dhc,hc->dh", Wr, a_d)
        out[f"Ws{i}"] = Ws
        out[f"Wd{i}"] = Wd
        out[f"W{i}"] = W
        f = h * c if i < 3 else c
        g = np.asarray(inp[f"g{i}"], np.float32)
        be = np.asarray(inp[f"be{i}"], np.float32)
        m = np.asarray(inp[f"m{i}"], np.float32)
        v = np.asarray(inp[f"v{i}"], np.float32)
        b = np.asarray(inp[f"b{i}"], np.float32)
        sc = g / np.sqrt(v + BNE)
        sh = be + (b - m) * sc
        out[f"SC{i}"] = np.tile(sc[None, :], (P, 1)).astype(np.float32)
        out[f"SH{i}"] = np.tile(sh[None, :], (P, 1)).astype(np.float32)
    # sentinel x column: x_sent @ Ws1 = SENT_HS for every head
    Ws1 = out["Ws1"]  # [128, 8]
    x_sent = np.linalg.pinv(Ws1.T) @ np.full(8, SENT_HS, np.float32)
    assert np.abs(x_sent @ Ws1 - SENT_HS).max() < 1e-3 * abs(SENT_HS)
    assert np.abs(x_sent).max() < 1e6
    out["x_sent"] = x_sent.astype(np.float32)
    out["W1S"] = np.ascontiguousarray(
        np.concatenate([out["W1"], out["Ws1"]], axis=1))          # [128, 1032]
    out["WD1"] = np.ascontiguousarray(out["Wd1"])                 # [128, 8]
    out["W2SD"] = np.ascontiguousarray(
        np.concatenate([out["W2"], out["Ws2"], out["Wd2"]], 1))   # [1024, 264]
    out["W3SD"] = np.ascontiguousarray(
        np.concatenate([out["W3"], out["Ws3"], out["Wd3"]], 1))   # [256, 34]
    out["WC1"] = np.asarray(inp["Wc1"], np.float32)
    out["WC2"] = np.asarray(inp["Wc2"], np.float32)
    out["BC1"] = np.tile(np.asarray(inp["bc1"], np.float32)[None, :], (P, 1))
    out["BC2"] = np.tile(np.asarray(inp["bc2"], np.float32)[None, :], (P, 1))
    return out


# ---------------------------------------------------------------- device side

def build_kernel(n, ncores, k_list, stage=3, debug_taps=False):
    nloc = n // ncores
    ntiles = len(k_list)
    offs = np.cumsum([0] + list(k_list))[:-1]
    nch = int(np.sum(k_list))
    kmax = max(k_list)

    nc = bacc.Bacc("TRN2", target_bir_lowering=False, debug=False,
                   num_devices=ncores)

    xt_d = nc.dram_tensor("XT", [P, n + 1], F32, kind="ExternalInput")
    xloct_d = nc.dram_tensor("XLOCT", [P, nloc], F32R, kind="ExternalInput")
    idx1_d = nc.dram_tensor("IDX1", [P, nch * 8], I16, kind="ExternalInput")
    idxg_d = nc.dram_tensor("IDXG", [P, nch * 8], I16, kind="ExternalInput")
    ident_d = nc.dram_tensor("IDENT", [P, P], F32R, kind="ExternalInput")
    w1s_d = nc.dram_tensor("W1S", [P, 1032], F32R, kind="ExternalInput")
    wd1_d = nc.dram_tensor("WD1", [P, 8], F32R, kind="ExternalInput")
    w2sd_d = nc.dram_tensor("W2SD", [1024, 264], F32R, kind="ExternalInput")
    w3sd_d = nc.dram_tensor("W3SD", [256, 34], F32R, kind="ExternalInput")
    wc1_d = nc.dram_tensor("WC1", [32, 16], F32R, kind="ExternalInput")
    wc2_d = nc.dram_tensor("WC2", [16, 2], F32R, kind="ExternalInput")
    wf_d = {}
    for nm, shp in (("SC1", [P, 1024]), ("SH1", [P, 1024]),
                    ("SC2", [P, 256]), ("SH2", [P, 256]),
                    ("SC3", [P, 32]), ("SH3", [P, 32]),
                    ("BC1", [P, 16]), ("BC2", [P, 2])):
        wf_d[nm] = nc.dram_tensor(nm, shp, F32, kind="ExternalInput")

    out_d = nc.dram_tensor("OUT", [nloc, 2], F32, kind="ExternalOutput")
    if debug_taps:
        dbg1_d = nc.dram_tensor("DBG1", [nloc, 264], F32, kind="ExternalOutput")
        dbg2_d = nc.dram_tensor("DBG2", [nloc, 34], F32, kind="ExternalOutput")

    shared = "Shared" if ncores > 1 else "Local"
    nlocp = nloc + 1  # + per-core sentinel row
    ag2_in = nc.dram_tensor("ag2_in", [nlocp, ROW2], F32)
    ag2_out = nc.dram_tensor("ag2_out", [nlocp * ncores, ROW2], F32,
                             addr_space=shared)
    ag3_in = nc.dram_tensor("ag3_in", [nlocp, ROW3], F32)
    ag3_out = nc.dram_tensor("ag3_out", [nlocp * ncores, ROW3], F32,
                             addr_space=shared)

    def rows_of(t):
        return min(P, nloc - t * P)

    with ExitStack() as ctx:
        tc = ctx.enter_context(tile.TileContext(nc))
        cpool = ctx.enter_context(tc.tile_pool(name="const", bufs=1))

        identr = cpool.tile([P, P], F32R)
        nc.sync.dma_start(out=identr[:], in_=ident_d[:, :])
        w1s = cpool.tile([P, 1032], F32R)
        nc.sync.dma_start(out=w1s[:], in_=w1s_d[:, :])
        wd1 = cpool.tile([P, 8], F32R)
        nc.sync.dma_start(out=wd1[:], in_=wd1_d[:, :])
        w2sd = cpool.tile([P, 8, 264], F32R)
        nc.sync.dma_start(out=w2sd[:],
                          in_=w2sd_d[:, :].rearrange("(kb p) f -> p kb f", p=P))
        w3sd = cpool.tile([P, 2, 34], F32R)
        nc.sync.dma_start(out=w3sd[:],
                          in_=w3sd_d[:, :].rearrange("(kb p) f -> p kb f", p=P))
        wc1 = cpool.tile([32, 16], F32R)
        nc.sync.dma_start(out=wc1[:], in_=wc1_d[:, :])
        wc2 = cpool.tile([16, 2], F32R)
        nc.sync.dma_start(out=wc2[:], in_=wc2_d[:, :])
        wf = {}
        for nm in ("SC1", "SH1", "SC2", "SH2", "SC3", "SH3", "BC1", "BC2"):
            wf[nm] = cpool.tile(list(wf_d[nm].shape), F32, name=nm)
            nc.sync.dma_start(out=wf[nm][:], in_=wf_d[nm][:, :])
        idx1 = cpool.tile([P, nch * 8], I16, name="idx1")
        nc.sync.dma_start(out=idx1[:], in_=idx1_d[:, :])
        idxg = cpool.tile([P, nch * 8], I16, name="idxg")
        nc.sync.dma_start(out=idxg[:], in_=idxg_d[:, :])

        hd1_sb = cpool.tile([P, ntiles * 8], F32, name="hd1")
        nc.vector.memset(hd1_sb[:], 0.0)
        hd2_sb = cpool.tile([P, ntiles * 4], F32, name="hd2")
        hd3_sb = cpool.tile([P, ntiles], F32, name="hd3")

        # per-core sentinel rows of the AllGather inputs: [0.. | hs=-1e3 | ..]
        sent2 = cpool.tile([1, ROW2], F32, name="sent2")
        nc.vector.memset(sent2[:], 0.0)
        nc.vector.memset(sent2[:, 256:260], SENT_HS)
        nc.sync.dma_start(out=ag2_in[nloc:nloc + 1, :], in_=sent2[:])
        sent3 = cpool.tile([1, ROW3], F32, name="sent3")
        nc.vector.memset(sent3[:], 0.0)
        nc.vector.memset(sent3[:, 32:33], SENT_HS)
        nc.sync.dma_start(out=ag3_in[nloc:nloc + 1, :], in_=sent3[:])

        # ---- phase 0: hd1 for local nodes (perm order)
        with tc.tile_pool(name="p0", bufs=1) as pool0, \
             tc.tile_pool(name="p0p", bufs=2, space="PSUM") as pp0:
            xloct = pool0.tile([P, nloc], F32R)
            nc.sync.dma_start(out=xloct[:], in_=xloct_d[:, :])
            for t in range(ntiles):
                r = rows_of(t)
                psD = pp0.tile([P, 8], F32, tag="psD")
                nc.tensor.matmul(psD[:r], lhsT=xloct[:, t * P:t * P + r],
                                 rhs=wd1[:], start=True, stop=True)
                nc.vector.tensor_copy(out=hd1_sb[:r, t * 8:(t + 1) * 8],
                                      in_=psD[:r])

        # ================= layer 1 =================
        with tc.tile_pool(name="xtp", bufs=1) as xtp, \
             tc.tile_pool(name="L1g", bufs=2) as gpool, \
             tc.tile_pool(name="L1s", bufs=2) as spool, \
             tc.tile_pool(name="L1v", bufs=2) as vpool, \
             tc.tile_pool(name="L1e", bufs=2) as epool, \
             tc.tile_pool(name="L1pS", bufs=1, space="PSUM") as ppS, \
             tc.tile_pool(name="L1pH", bufs=2, space="PSUM") as ppH, \
             tc.tile_pool(name="L1pA", bufs=1, space="PSUM") as ppA:
            xt = xtp.tile([P, n + 1], F32)
            nc.sync.dma_start(out=xt[:], in_=xt_d[:, :])
            for t in range(ntiles):
                k = k_list[t]
                off = int(offs[t])
                r = rows_of(t)
                xg = gpool.tile([P, kmax * P], F32, tag="xg")
                xgr = gpool.tile([P, kmax * P], F32R, tag="xgr")
                for j in range(k):
                    nc.gpsimd.ap_gather(
                        out_ap=xg[:, j * P:(j + 1) * P], in_ap=xt[:],
                        idxs_ap=idx1[:, (off + j) * 8:(off + j + 1) * 8],
                        channels=P, num_elems=n + 1, d=1, num_idxs=P)
                    nc.scalar.copy(out=xgr[:, j * P:(j + 1) * P],
                                   in_=xg[:, j * P:(j + 1) * P])
                psS = ppS.tile([P, kmax * 8], F32, tag="psS")
                for j in range(k):
                    nc.tensor.matmul(psS[:, j * 8:(j + 1) * 8],
                                     lhsT=xgr[:, j * P:(j + 1) * P],
                                     rhs=w1s[:, 1024:1032],
                                     start=True, stop=True)
                # softmax over slots, [p, h, j] layout
                sc = spool.tile([P, kmax * 8], F32, tag="sc")
                nc.vector.tensor_tensor(
                    out=sc[:, :8 * k].rearrange("p (h j) -> p h j", j=k),
                    in0=psS[:, :8 * k].rearrange("p (j h) -> p h j", h=8),
                    in1=hd1_sb[:, t * 8:(t + 1) * 8][:, :, None]
                        .to_broadcast([P, 8, k]),
                    op=OP.add)
                sc2 = spool.tile([P, kmax * 8], F32, tag="sc2")
                nc.vector.tensor_scalar(out=sc2[:, :8 * k], in0=sc[:, :8 * k],
                                        scalar1=LEAK, scalar2=None, op0=OP.mult)
                nc.vector.tensor_tensor(out=sc2[:, :8 * k], in0=sc[:, :8 * k],
                                        in1=sc2[:, :8 * k], op=OP.max)
                expsc = spool.tile([P, kmax * 8], F32, tag="expsc")
                nc.scalar.activation(out=expsc[:, :8 * k], in_=sc2[:, :8 * k],
                                     func=AF.Exp)
                z = spool.tile([P, 8], F32, tag="z")
                nc.vector.tensor_reduce(
                    out=z[:], in_=expsc[:, :8 * k].rearrange(
                        "p (h j) -> p h j", j=k),
                    axis=AX.X, op=OP.add)
                nc.vector.tensor_scalar(out=z[:], in0=z[:], scalar1=float(EPS),
                                        scalar2=None, op0=OP.add)
                zr = spool.tile([P, 8], F32, tag="zr")
                with nc.allow_low_precision(reason="softmax 1/Z"):
                    nc.vector.reciprocal(out=zr[:], in_=z[:])
                alpha = spool.tile([P, kmax * 8], F32, tag="alpha")
                nc.vector.tensor_tensor(
                    out=alpha[:, :8 * k].rearrange("p (h j) -> p h j", j=k),
                    in0=expsc[:, :8 * k].rearrange("p (h j) -> p h j", j=k),
                    in1=zr[:, :, None].to_broadcast([P, 8, k]),
                    op=OP.mult)
                # weighted aggregation
                pacc = ppA.tile([P, 1024], F32, tag="acc")
                for j in range(k):
                    for q in range(2):
                        qs = slice(q * 512, (q + 1) * 512)
                        psH = ppH.tile([P, 512], F32, tag="h")
                        nc.tensor.matmul(psH[:],
                                         lhsT=xgr[:, j * P:(j + 1) * P],
                                         rhs=w1s[:, qs], start=True, stop=True)
                        v = vpool.tile([P, 512], F32R, tag="v")
                        nc.vector.tensor_tensor(
                            out=v[:].rearrange("p (h c) -> p h c", c=P),
                            in0=psH[:].rearrange("p (h c) -> p h c", c=P),
                            in1=alpha[:, q * 4 * k + j:8 * k:k][:, :4, None]
                                .to_broadcast([P, 4, P]),
                            op=OP.mult)
                        nc.tensor.matmul(pacc[:, qs], lhsT=identr[:],
                                         rhs=v[:],
                                         start=(j == 0), stop=(j == k - 1))
                # ---- epilogue 1: x2 = relu(bn(acc)); [h2|hs2] -> ag2_in
                x2 = epool.tile([P, 1024], F32R, tag="x2")
                nc.vector.tensor_tensor(out=x2[:], in0=pacc[:],
                                        in1=wf["SC1"][:], op=OP.mult)
                nc.vector.tensor_tensor(out=x2[:], in0=x2[:],
                                        in1=wf["SH1"][:], op=OP.add)
                nc.scalar.activation(out=x2[:], in_=x2[:], func=AF.Relu)
                psW = ppS.tile([P, 264], F32, tag="pw")
                for rr in range(8):
                    tp = ppS.tile([P, P], F32R, tag="tp")
                    nc.tensor.transpose(tp[:], x2[:, rr * P:(rr + 1) * P],
                                        identr[:])
                    x2T = epool.tile([P, P], F32R, tag="x2T")
                    if rr % 2 == 0:
                        nc.scalar.copy(out=x2T[:], in_=tp[:])
                    else:
                        nc.vector.tensor_copy(out=x2T[:], in_=tp[:])
                    nc.tensor.matmul(psW[:], lhsT=x2T[:], rhs=w2sd[:, rr, :],
                                     start=(rr == 0), stop=(rr == 7))
                agrow = epool.tile([P, 264], F32, tag="agrow")
                nc.vector.tensor_copy(out=agrow[:], in_=psW[:])
                nc.sync.dma_start(out=ag2_in[t * P:t * P + r, 0:260],
                                  in_=agrow[:r, 0:260])
                nc.vector.tensor_copy(out=hd2_sb[:, t * 4:(t + 1) * 4],
                                      in_=agrow[:, 260:264])
                if debug_taps:
                    nc.sync.dma_start(out=dbg1_d[t * P:t * P + r, :],
                                      in_=agrow[:r, :])

        if stage >= 2 and ncores > 1:
            nc.gpsimd.collective_compute(
                "AllGather", OP.bypass,
                replica_groups=[list(range(ncores))],
                ins=[ag2_in[:, :]], outs=[ag2_out[:, :]])
        elif stage >= 2:
            nc.sync.dma_start(out=ag2_out[:, :], in_=ag2_in[:, :])

        # ================= layer 2 =================
        with tc.tile_pool(name="L2g", bufs=2) as gpool, \
             tc.tile_pool(name="L2s", bufs=2) as spool, \
             tc.tile_pool(name="L2v", bufs=2) as vpool, \
             tc.tile_pool(name="L2e", bufs=2) as epool, \
             tc.tile_pool(name="L2pS", bufs=2, space="PSUM") as ppS, \
             tc.tile_pool(name="L2pA", bufs=2, space="PSUM") as ppA:
            for t in range(ntiles if stage >= 2 and not os.environ.get('SKIP_L2') else 0):
                k = k_list[t]
                off = int(offs[t])
                r = rows_of(t)
                hg = gpool.tile([P, kmax * ROW2], F32, tag="hg")
                for j0 in range(0, k, 2):
                    kk = min(2, k - j0)
                    nc.gpsimd.dma_gather(
                        out_ap=hg[:, j0 * ROW2:(j0 + kk) * ROW2].rearrange(
                            "p (k f) -> p k f", f=ROW2),
                        in_ap=ag2_out[:, :],
                        idxs_ap=idxg[:, (off + j0) * 8:(off + j0 + kk) * 8],
                        num_idxs=kk * P, num_idxs_reg=kk * P, elem_size=ROW2)
                hg3 = hg[:, :k * ROW2].rearrange("p (k f) -> p k f", f=ROW2)
                sc = spool.tile([P, kmax * 4], F32, tag="sc")
                nc.vector.tensor_tensor(
                    out=sc[:, :4 * k].rearrange("p (h j) -> p h j", j=k),
                    in0=hg3[:, :, 256:260].rearrange("p j h -> p h j"),
                    in1=hd2_sb[:, t * 4:(t + 1) * 4][:, :, None]
                        .to_broadcast([P, 4, k]),
                    op=OP.add)
                sc2 = spool.tile([P, kmax * 4], F32, tag="sc2")
                nc.vector.tensor_scalar(out=sc2[:, :4 * k], in0=sc[:, :4 * k],
                                        scalar1=LEAK, scalar2=None, op0=OP.mult)
                nc.vector.tensor_tensor(out=sc2[:, :4 * k], in0=sc[:, :4 * k],
                                        in1=sc2[:, :4 * k], op=OP.max)
                expsc = spool.tile([P, kmax * 4], F32, tag="expsc")
                nc.scalar.activation(out=expsc[:, :4 * k], in_=sc2[:, :4 * k],
                                     func=AF.Exp)
                z = spool.tile([P, 4], F32, tag="z")
                nc.vector.tensor_reduce(
                    out=z[:], in_=expsc[:, :4 * k].rearrange(
                        "p (h j) -> p h j", j=k),
                    axis=AX.X, op=OP.add)
                nc.vector.tensor_scalar(out=z[:], in0=z[:], scalar1=float(EPS),
                                        scalar2=None, op0=OP.add)
                zr = spool.tile([P, 4], F32, tag="zr")
                with nc.allow_low_precision(reason="softmax 1/Z"):
                    nc.vector.reciprocal(out=zr[:], in_=z[:])
                alpha = spool.tile([P, kmax * 4], F32, tag="alpha")
                nc.vector.tensor_tensor(
                    out=alpha[:, :4 * k].rearrange("p (h j) -> p h j", j=k),
                    in0=expsc[:, :4 * k].rearrange("p (h j) -> p h j", j=k),
                    in1=zr[:, :, None].to_broadcast([P, 4, k]),
                    op=OP.mult)
                v = vpool.tile([P, kmax * 256], F32R, tag="v")
                nc.vector.tensor_tensor(
                    out=v[:, :k * 256].rearrange(
                        "p (j h c) -> p j h c", h=4, c=64),
                    in0=hg3[:, :, 0:256].rearrange(
                        "p j (h c) -> p j h c", c=64),
                    in1=alpha[:, :4 * k].rearrange(
                        "p (h j) -> p j h", j=k)[:, :, :, None]
                        .to_broadcast([P, k, 4, 64]),
                    op=OP.mult)
                pacc = ppA.tile([P, 256], F32, tag="acc")
                for j in range(k):
                    nc.tensor.matmul(pacc[:], lhsT=identr[:],
                                     rhs=v[:, j * 256:(j + 1) * 256],
                                     start=(j == 0), stop=(j == k - 1))
                # ---- epilogue 2
                x3 = epool.tile([P, 256], F32R, tag="x3")
                nc.vector.tensor_tensor(out=x3[:], in0=pacc[:],
                                        in1=wf["SC2"][:], op=OP.mult)
                nc.vector.tensor_tensor(out=x3[:], in0=x3[:],
                                        in1=wf["SH2"][:], op=OP.add)
                nc.scalar.activation(out=x3[:], in_=x3[:], func=AF.Relu)
                psW = ppA.tile([P, 34], F32, tag="pw")
                for rr in range(2):
                    tp = ppS.tile([P, P], F32R, tag="tp")
                    nc.tensor.transpose(tp[:], x3[:, rr * P:(rr + 1) * P],
                                        identr[:])
                    x3T = epool.tile([P, P], F32R, tag="x3T")
                    nc.vector.tensor_copy(out=x3T[:], in_=tp[:])
                    nc.tensor.matmul(psW[:], lhsT=x3T[:], rhs=w3sd[:, rr, :],
                                     start=(rr == 0), stop=(rr == 1))
                agrow = epool.tile([P, 34], F32, tag="agrow")
                nc.vector.tensor_copy(out=agrow[:], in_=psW[:])
                nc.sync.dma_start(out=ag3_in[t * P:t * P + r, 0:33],
                                  in_=agrow[:r, 0:33])
                nc.vector.tensor_copy(out=hd3_sb[:, t:t + 1],
                                      in_=agrow[:, 33:34])

        if stage >= 3 and ncores > 1:
            nc.gpsimd.collective_compute(
                "AllGather", OP.bypass,
                replica_groups=[list(range(ncores))],
                ins=[ag3_in[:, :]], outs=[ag3_out[:, :]])
        elif stage >= 3:
            nc.sync.dma_start(out=ag3_out[:, :], in_=ag3_in[:, :])

        # ================= layer 3 (+ classifier) =================
        with tc.tile_pool(name="L3g", bufs=2) as gpool, \
             tc.tile_pool(name="L3s", bufs=2) as spool, \
             tc.tile_pool(name="L3e", bufs=2) as epool, \
             tc.tile_pool(name="L3p", bufs=2, space="PSUM") as pp3:
            for t in range(ntiles if stage >= 3 else 0):
                k = k_list[t]
                off = int(offs[t])
                r = rows_of(t)
                hg = gpool.tile([P, kmax * ROW3], F32, tag="hg")
                for j0 in range(0, k, 8):
                    kk = min(8, k - j0)
                    nc.gpsimd.dma_gather(
                        out_ap=hg[:, j0 * ROW3:(j0 + kk) * ROW3].rearrange(
                            "p (k f) -> p k f", f=ROW3),
                        in_ap=ag3_out[:, :],
                        idxs_ap=idxg[:, (off + j0) * 8:(off + j0 + kk) * 8],
                        num_idxs=kk * P, num_idxs_reg=kk * P, elem_size=ROW3)
                hg3 = hg[:, :k * ROW3].rearrange("p (k f) -> p k f", f=ROW3)
                sc = spool.tile([P, kmax], F32, tag="sc")
                nc.vector.tensor_scalar(
                    out=sc[:, :k], in0=hg3[:, :, 32],
                    scalar1=hd3_sb[:, t:t + 1], scalar2=None, op0=OP.add)
                sc2 = spool.tile([P, kmax], F32, tag="sc2")
                nc.vector.tensor_scalar(out=sc2[:, :k], in0=sc[:, :k],
                                        scalar1=LEAK, scalar2=None, op0=OP.mult)
                nc.vector.tensor_tensor(out=sc2[:, :k], in0=sc[:, :k],
                                        in1=sc2[:, :k], op=OP.max)
                expsc = spool.tile([P, kmax], F32, tag="expsc")
                z = spool.tile([P, 1], F32, tag="z")
                nc.scalar.activation(out=expsc[:, :k], in_=sc2[:, :k],
                                     func=AF.Exp, accum_out=z[:])
                nc.vector.tensor_scalar(out=z[:], in0=z[:], scalar1=float(EPS),
                                        scalar2=None, op0=OP.add)
                zr = spool.tile([P, 1], F32, tag="zr")
                with nc.allow_low_precision(reason="softmax 1/Z"):
                    nc.vector.reciprocal(out=zr[:], in_=z[:])
                alpha = spool.tile([P, kmax], F32, tag="alpha")
                nc.vector.tensor_scalar(out=alpha[:, :k], in0=expsc[:, :k],
                                        scalar1=zr[:, 0:1], scalar2=None,
                                        op0=OP.mult)
                v = spool.tile([P, kmax * 32], F32, tag="v")
                nc.vector.tensor_tensor(
                    out=v[:, :k * 32].rearrange("p (j c) -> p j c", c=32),
                    in0=hg3[:, :, 0:32],
                    in1=alpha[:, :k, None].to_broadcast([P, k, 32]),
                    op=OP.mult)
                acc3 = spool.tile([P, 32], F32, tag="acc3")
                nc.vector.tensor_reduce(
                    out=acc3[:], in_=v[:, :k * 32].rearrange(
                        "p (j c) -> p c j", c=32),
                    axis=AX.X, op=OP.add)
                # ---- epilogue 3: bn+relu, classifier, log_softmax
                x4 = epool.tile([P, 32], F32R, tag="x4")
                nc.vector.tensor_tensor(out=x4[:], in0=acc3[:],
                                        in1=wf["SC3"][:], op=OP.mult)
                nc.vector.tensor_tensor(out=x4[:], in0=x4[:],
                                        in1=wf["SH3"][:], op=OP.add)
                nc.scalar.activation(out=x4[:], in_=x4[:], func=AF.Relu)
                tp = pp3.tile([P, P], F32R, tag="tp")
                nc.tensor.transpose(tp[0:32, :], x4[:], identr[:])
                x4T = epool.tile([32, P], F32R, tag="x4T")
                nc.vector.tensor_copy(out=x4T[:], in_=tp[0:32, :])
                z1p = pp3.tile([P, 16], F32, tag="z1p")
                nc.tensor.matmul(z1p[:], lhsT=x4T[:], rhs=wc1[:],
                                 start=True, stop=True)
                z1 = epool.tile([P, 16], F32R, tag="z1")
                nc.vector.tensor_tensor(out=z1[:], in0=z1p[:],
                                        in1=wf["BC1"][:], op=OP.add)
                nc.scalar.activation(out=z1[:], in_=z1[:], func=AF.Relu)
                tp2 = pp3.tile([P, P], F32R, tag="tp2")
                nc.tensor.transpose(tp2[0:16, :], z1[:], identr[:])
                z1T = epool.tile([16, P], F32R, tag="z1T")
                nc.vector.tensor_copy(out=z1T[:], in_=tp2[0:16, :])
                z2p = pp3.tile([P, 2], F32, tag="z2p")
                nc.tensor.matmul(z2p[:], lhsT=z1T[:], rhs=wc2[:],
                                 start=True, stop=True)
                z2 = epool.tile([P, 2], F32, tag="z2")
                nc.vector.tensor_tensor(out=z2[:], in0=z2p[:],
                                        in1=wf["BC2"][:], op=OP.add)
                ez = epool.tile([P, 2], F32, tag="ez")
                lse = epool.tile([P, 1], F32, tag="lse")
                nc.scalar.activation(out=ez[:], in_=z2[:], func=AF.Exp,
                                     accum_out=lse[:])
                nc.scalar.activation(out=lse[:], in_=lse[:], func=AF.Ln)
                res = epool.tile([P, 2], F32, tag="res")
                nc.vector.tensor_scalar(out=res[:], in0=z2[:],
                                        scalar1=lse[:, 0:1], scalar2=None,
                                        op0=OP.subtract)
                nc.sync.dma_start(out=out_d[t * P:t * P + r, :], in_=res[:r])

    nc.compile()
    return nc


# ---------------------------------------------------------------- entry point

_CACHE = {}


def make_in_maps(inputs, ncores=NCORES):
    edge_index = np.asarray(inputs["edge_index"])
    x = np.asarray(inputs["x"], dtype=np.float32)
    n = x.shape[0]
    nloc = n // ncores
    k_list, IDX1, IDXG, perms, devrow = _preprocess(edge_index, n, ncores)
    w = _fold_weights(inputs)

    xt = np.concatenate([x.T, w["x_sent"][:, None]], axis=1)  # [128, n+1]
    ident = np.ascontiguousarray(np.eye(P, dtype=np.float32))
    base = dict(XT=np.ascontiguousarray(xt), IDENT=ident)
    for nm in ("W1S", "WD1", "W2SD", "W3SD", "WC1", "WC2",
               "SC1", "SH1", "SC2", "SH2", "SC3", "SH3", "BC1", "BC2"):
        base[nm] = np.ascontiguousarray(w[nm])
    in_maps = []
    for c in range(ncores):
        m = dict(base)
        loc = x[c * nloc + perms[c]]
        m["XLOCT"] = np.ascontiguousarray(loc.T)
        m["IDX1"] = np.ascontiguousarray(IDX1[c])
        m["IDXG"] = np.ascontiguousarray(IDXG[c])
        in_maps.append(m)
    return n, k_list, in_maps, devrow


def kernel(**inputs):
    n, k_list, in_maps, devrow = make_in_maps(inputs)
    key = (n, tuple(k_list))
    if key not in _CACHE:
        _CACHE[key] = build_kernel(n, NCORES, k_list)
    nc = _CACHE[key]
    res = run_bass_kernel_spmd(nc, in_maps, core_ids=list(range(NCORES)))
    allout = np.concatenate([r["OUT"] for r in res.results], axis=0)
    return allout[devrow].astype(np.float32)
